# revision 29
# baseline (speedup 1.0000x reference)
# nn_GRUEncoder: B=256, T=512, IN=75, H=256, 2-layer GRU + fc.
# Data-parallel over 8 NeuronCores (32 batch rows each). Full inputs in,
# full output out.
#
# Per-core Bass/Tile kernel design:
#  - All tensors live "transposed": hidden/gate dims on SBUF partitions,
#    batch (32) on the free dim. fp16 matmul operands, fp32 PSUM/DVE math.
#  - GRU state is stored offset: ht = h + 1 (so h0=0 -> ht=1). With
#    n = tanh(p) = 2*sigmoid(2p) - 1 and doubled n-gate weights, the whole
#    per-step elementwise chain is sigmoid-only:
#        d = ht - 2s        (= h - n)       [scalar_tensor_tensor]
#        e = z * d                          [tensor_tensor]
#        ht' = 2s + e       (= h' + 1)      [scalar_tensor_tensor]
#    The -rowsum(W) corrections from the +1 offset and all biases fold into
#    weight-augmentation rows / eviction biases at host-prep time.
#  - Layer-0 input projection xg0 runs per-step (aug row in x carries all
#    layer-0 psumA biases). Layer-1 input projection xg1 runs as a bulk
#    GEMM per 16-step sub-chunk into an SBUF ring (bf16/fp16, no DRAM).
#  - The two layers run as a staggered wavefront: while layer 0 processes
#    chunk c, layer 1 processes chunk c-1, so their serial chains hide
#    each other and every engine stays busy.

import sys

sys.path.insert(0, "/opt/trn_rl_repo")

import numpy as np

P, B, H, G, K0, CH, T = 128, 32, 256, 768, 76, 32, 512
NCH = T // CH
NCORES = 8

_NC_CACHE = {}


def _build(T_=T):
    import concourse.bass as bass
    import concourse.tile as tile
    from concourse import mybir
    from concourse.bass import ds, ts

    f16 = mybir.dt.float16
    f32 = mybir.dt.float32
    AF = mybir.ActivationFunctionType
    OP = mybir.AluOpType
    NCH_ = T_ // CH

    from concourse import bacc

    nc = bacc.Bacc(None, target_bir_lowering=False)
    x_d = nc.dram_tensor("x", [K0, T_ * B], f16, kind="ExternalInput")
    wih0_d = nc.dram_tensor("wih0", [K0, G], f16, kind="ExternalInput")
    whh0_d = nc.dram_tensor("whh0", [P, 2 * G], f16, kind="ExternalInput")
    wih1_d = nc.dram_tensor("wih1", [P, 2 * G], f16, kind="ExternalInput")
    whh1_d = nc.dram_tensor("whh1", [P, 2 * G], f16, kind="ExternalInput")
    ident_d = nc.dram_tensor("ident", [P, P], f16, kind="ExternalInput")
    bias0B_d = nc.dram_tensor("bias0B", [P, 2 * B], f16, kind="ExternalInput")
    bias1A_d = nc.dram_tensor("bias1A", [P, 6], f32, kind="ExternalInput")
    bias1B_d = nc.dram_tensor("bias1B", [P, 2 * B], f16, kind="ExternalInput")
    fcw_d = nc.dram_tensor("fcw", [P, 2 * H], f16, kind="ExternalInput")
    fcb_d = nc.dram_tensor("fcb", [1, H], f16, kind="ExternalInput")
    out_d = nc.dram_tensor("out", [2, P, B], f32, kind="ExternalOutput")

    with tile.TileContext(nc) as tc:
        from contextlib import ExitStack

        with ExitStack() as ctx:
            consts = ctx.enter_context(tc.tile_pool(name="consts", bufs=1))
            interm = ctx.enter_context(tc.tile_pool(name="interm", bufs=6))
            # PSUM: 8 banks total. Layer0: R/ZN/B banks (bufs=1 each);
            # Layer1: R/Z/B banks (bufs=1); bulk GEMM: 2 banks.
            psL0 = ctx.enter_context(tc.tile_pool(name="psL0", bufs=1, space="PSUM"))
            psL1 = ctx.enter_context(tc.tile_pool(name="psL1", bufs=1, space="PSUM"))
            psK = ctx.enter_context(tc.tile_pool(name="psK", bufs=2, space="PSUM"))

            def dep(a, b):
                # order-only edge: a must execute after b (same engine)
                tile.add_dep_helper(a.ins, b.ins, sync=False, reason="psum-group-order")

            wih0 = consts.tile([K0, G], f16)
            whh0 = consts.tile([P, 2 * G], f16)
            wih1 = consts.tile([P, 2 * G], f16)
            whh1 = consts.tile([P, 2 * G], f16)
            ident = consts.tile([P, P], f16)
            bias0B = consts.tile([P, 2, B], f16)
            bias1A = consts.tile([P, 6], f32)
            bias1B = consts.tile([P, 2, B], f16)
            fcw = consts.tile([P, 2 * H], f16)
            fcb = consts.tile([1, H], f16)
            nc.sync.dma_start(wih0[:], wih0_d[:])
            nc.sync.dma_start(whh0[:], whh0_d[:])
            nc.sync.dma_start(wih1[:], wih1_d[:])
            nc.sync.dma_start(whh1[:], whh1_d[:])
            nc.sync.dma_start(ident[:], ident_d[:])
            nc.sync.dma_start(bias0B[:], bias0B_d[:].rearrange("p (s b) -> p s b", b=B))
            nc.sync.dma_start(bias1A[:], bias1A_d[:])
            nc.sync.dma_start(bias1B[:], bias1B_d[:].rearrange("p (s b) -> p s b", b=B))
            nc.sync.dma_start(fcw[:], fcw_d[:])
            nc.sync.dma_start(fcb[:], fcb_d[:])

            ones = consts.tile([1, B], f16)
            nc.vector.memset(ones[:], 1.0)

            # x chunk staging ring (2 slots), per-chunk DMA from DRAM
            xr = [consts.tile([K0, CH * B], f16, tag=f"xr{i}", name=f"xr{i}") for i in range(2)]
            # layer-0 state ring: ht0 per step, [k-chunk, t, b]; doubles as
            # the moving operand for the bulk xg1 GEMM
            r0 = [consts.tile([P, 2, CH, B], f16, tag=f"r0{i}", name=f"r0{i}") for i in range(2)]
            # xg1 ring: [t, strip, b]
            r1 = [consts.tile([P, CH, 6, B], f16, tag=f"r1{i}", name=f"r1{i}") for i in range(2)]
            h1 = consts.tile([P, 2, B], f16)

            # ht init = 1 (h=0). Chunk 0 step 0 reads r0[1][:, :, CH-1, :].
            nc.vector.memset(r0[1][:, :, CH - 1, :], 1.0)
            nc.vector.memset(h1[:], 1.0)

            def l0_step(t, par, hp, hout):
                """One layer-0 GRU step. hp: [P,2,B] ht_{t-1}; hout: ring slot."""
                bR = psL0.tile([P, 2, B], f32, tag="bR0")   # pre-r (biased)
                bZN = psL0.tile([P, 4, B], f32, tag="bZN0")  # pre-z, 2*xn -> npre2
                bB = psL0.tile([P, 2, B], f32, tag="bB0")   # 2*(hn + b_hn)
                xs = xr[par][:, ts(t, B)]
                # --- bank R: 6 MMs, one group ---
                m0 = nc.tensor.matmul(bR[:, 0, :], wih0[:, 0:128], xs, start=True, stop=False)
                m1 = nc.tensor.matmul(bR[:, 1, :], wih0[:, 128:256], xs, start=False, stop=False)
                dep(m1, m0)
                last0 = last1 = None
                for k in (0, 1):
                    hk = hp[:, k, :]
                    last0 = nc.tensor.matmul(bR[:, 0, :], whh0[:, k * G + 0 : k * G + 128], hk, start=False, stop=False)
                    last1 = nc.tensor.matmul(bR[:, 1, :], whh0[:, k * G + 128 : k * G + 256], hk, start=False, stop=k == 1)
                dep(last1, last0)
                r = interm.tile([P, 2, B], f16, tag="r")
                nc.scalar.activation(r[:], bR[:], AF.Sigmoid)
                # --- bank ZN: z strips 0-1, n strips 2-3; one group ---
                z0 = nc.tensor.matmul(bZN[:, 0, :], wih0[:, 256:384], xs, start=True, stop=False)
                z1 = nc.tensor.matmul(bZN[:, 1, :], wih0[:, 384:512], xs, start=False, stop=False)
                n0 = nc.tensor.matmul(bZN[:, 2, :], wih0[:, 512:640], xs, start=False, stop=False)
                n1 = nc.tensor.matmul(bZN[:, 3, :], wih0[:, 640:768], xs, start=False, stop=False)
                for m in (z1, n0, n1):
                    dep(m, z0)
                lz0 = lz1 = None
                for k in (0, 1):
                    hk = hp[:, k, :]
                    lz0 = nc.tensor.matmul(bZN[:, 0, :], whh0[:, k * G + 256 : k * G + 384], hk, start=False, stop=False)
                    lz1 = nc.tensor.matmul(bZN[:, 1, :], whh0[:, k * G + 384 : k * G + 512], hk, start=False, stop=k == 1)
                for m in (lz0, n0, n1):
                    dep(lz1, m)
                # --- bank B: bias seed (identity matmul) first, then hg-n ---
                nc.tensor.matmul(bB[:], ident[:], bias0B[:], start=True, stop=False)
                lb0 = lb1 = None
                for k in (0, 1):
                    hk = hp[:, k, :]
                    lb0 = nc.tensor.matmul(bB[:, 0, :], whh0[:, k * G + 512 : k * G + 640], hk, start=False, stop=False)
                    lb1 = nc.tensor.matmul(bB[:, 1, :], whh0[:, k * G + 640 : k * G + 768], hk, start=False, stop=k == 1)
                dep(lb1, lb0)
                t1 = interm.tile([P, 2, B], f16, tag="t1")
                nc.vector.tensor_tensor(t1[:], r[:], bB[:], OP.mult)
                nc.vector.tensor_tensor(bZN[:, 2:4, :], bZN[:, 2:4, :], t1[:], OP.add)
                szn = interm.tile([P, 4, B], f16, tag="szn")
                nc.scalar.activation(szn[:], bZN[:], AF.Sigmoid)
                d = interm.tile([P, 2, B], f16, tag="d")
                nc.vector.scalar_tensor_tensor(d[:], szn[:, 2:4, :], -2.0, hp, OP.mult, OP.add)
                e = interm.tile([P, 2, B], f16, tag="e")
                nc.vector.tensor_tensor(e[:], szn[:, 0:2, :], d[:], OP.mult)
                nc.vector.scalar_tensor_tensor(hout, szn[:, 2:4, :], 2.0, e[:], OP.mult, OP.add)

            def l1_step(t, par):
                """One layer-1 GRU step. xg1 + biases enter the step PSUM banks
                via identity-matmul seeds (cheap, dependency-free PE work);
                hg accumulates on top. State h1 updated in place."""
                bR = psL1.tile([P, 2, B], f32, tag="bR1")
                bZN = psL1.tile([P, 4, B], f32, tag="bZN1")
                bB = psL1.tile([P, 2, B], f32, tag="bB1")
                rg = r1[par]
                # seeds (full-bank writes; later strip MMs auto-order after them)
                nc.tensor.matmul(bR[:], ident[:], rg[:, t, 0:2, :], start=True, stop=False)
                nc.tensor.matmul(bZN[:], ident[:], rg[:, t, 2:6, :], start=True, stop=False)
                nc.tensor.matmul(bB[:], ident[:], bias1B[:], start=True, stop=False)
                # bank R: + hg-r
                l0_ = l1_ = None
                for k in (0, 1):
                    hk = h1[:, k, :]
                    l0_ = nc.tensor.matmul(bR[:, 0, :], whh1[:, k * G + 0 : k * G + 128], hk, start=False, stop=False)
                    l1_ = nc.tensor.matmul(bR[:, 1, :], whh1[:, k * G + 128 : k * G + 256], hk, start=False, stop=k == 1)
                dep(l1_, l0_)
                r = interm.tile([P, 2, B], f16, tag="r1t")
                nc.scalar.activation(r[:], bR[:], AF.Sigmoid)
                # bank B: + hg-n
                lb0 = lb1 = None
                for k in (0, 1):
                    hk = h1[:, k, :]
                    lb0 = nc.tensor.matmul(bB[:, 0, :], whh1[:, k * G + 512 : k * G + 640], hk, start=False, stop=False)
                    lb1 = nc.tensor.matmul(bB[:, 1, :], whh1[:, k * G + 640 : k * G + 768], hk, start=False, stop=k == 1)
                dep(lb1, lb0)
                # bank ZN: + hg-z into strips 0-1
                lz0 = lz1 = None
                for k in (0, 1):
                    hk = h1[:, k, :]
                    lz0 = nc.tensor.matmul(bZN[:, 0, :], whh1[:, k * G + 256 : k * G + 384], hk, start=False, stop=False)
                    lz1 = nc.tensor.matmul(bZN[:, 1, :], whh1[:, k * G + 384 : k * G + 512], hk, start=False, stop=k == 1)
                dep(lz1, lz0)
                t1 = interm.tile([P, 2, B], f16, tag="t1b")
                nc.vector.tensor_tensor(t1[:], r[:], bB[:], OP.mult)
                nc.vector.tensor_tensor(bZN[:, 2:4, :], bZN[:, 2:4, :], t1[:], OP.add)
                szn = interm.tile([P, 4, B], f16, tag="szn1")
                nc.scalar.activation(szn[:], bZN[:], AF.Sigmoid)
                # tail on GpSimd (tensor_tensor only - STT unsupported on Pool)
                s2 = interm.tile([P, 2, B], f16, tag="s2b")
                nc.vector.tensor_scalar(s2[:], szn[:, 2:4, :], 2.0, None, OP.mult)
                d = interm.tile([P, 2, B], f16, tag="d1")
                nc.gpsimd.tensor_tensor(d[:], h1[:, :, :], s2[:], OP.subtract)
                e = interm.tile([P, 2, B], f16, tag="e1")
                nc.gpsimd.tensor_tensor(e[:], szn[:, 0:2, :], d[:], OP.mult)
                nc.gpsimd.tensor_tensor(h1[:, :, :], s2[:], e[:], OP.add)

            def bulk_pieces(par, u):
                """xg1 bulk GEMM for 16 steps (sub-chunk u of the chunk in
                ring slot par): 6 strips x (2 matmuls + biased evict).
                Returns a list of closures to spread across step emission."""
                ops = []
                for s in range(6):
                    hold = {}

                    def mm0(s=s, hold=hold):
                        bp = psK.tile([P, 512], f32, tag="bp")
                        hold["bp"] = bp
                        nc.tensor.matmul(
                            bp[:], wih1[:, s * 128 : (s + 1) * 128],
                            r0[par][:, 0, u * 16 : (u + 1) * 16, :], start=True, stop=False)

                    def mm1(s=s, hold=hold):
                        bp = hold["bp"]
                        nc.tensor.matmul(
                            bp[:], wih1[:, G + s * 128 : G + (s + 1) * 128],
                            r0[par][:, 1, u * 16 : (u + 1) * 16, :], start=False, stop=True)

                    def ev(s=s, hold=hold):
                        bp = hold["bp"]
                        nc.vector.tensor_scalar(
                            r1[par][:, u * 16 : (u + 1) * 16, s, :],
                            bp[:].rearrange("p (t b) -> p t b", b=B),
                            bias1A[:, s : s + 1], None, OP.add)

                    ops += [mm0, mm1, ev]
                return ops

            def emit_half(par, xoff_next, with_l1, with_l0=True, last_l1_par=None):
                """Process L0 chunk (parity par) + L1 chunk (parity 1-par)."""
                if xoff_next is not None:
                    nc.sync.dma_start(xr[1 - par][:], x_d[:, ds(xoff_next, CH * B)])
                u1p = bulk_pieces(1 - par, 1) if with_l1 else []
                u0p = bulk_pieces(par, 0) if with_l0 else []
                n1, n0 = len(u1p), len(u0p)
                for t in range(CH):
                    if with_l0:
                        hp = r0[par][:, :, t - 1, :] if t > 0 else r0[1 - par][:, :, CH - 1, :]
                        l0_step(t, par, hp, r0[par][:, :, t, :])
                    if with_l1:
                        l1_step(t, 1 - par)
                    # spread prev-chunk sub-1 bulk over t=0..15
                    if t < 16:
                        for i in range(t * n1 // 16, (t + 1) * n1 // 16):
                            u1p[i]()
                    else:
                        for i in range((t - 16) * n0 // 16, (t - 15) * n0 // 16):
                            u0p[i]()

            def fc_emit():
                pf = psK.tile([P, 2, B], f32, tag="bp")
                first = None
                lasts = []
                for s in (0, 1):
                    for k in (0, 1):
                        m = nc.tensor.matmul(
                            pf[:, s, :], fcw[:, k * H + s * 128 : k * H + (s + 1) * 128],
                            h1[:, k, :], start=first is None, stop=False)
                        if first is None:
                            first = m
                        else:
                            dep(m, first)
                    m = nc.tensor.matmul(pf[:, s, :], fcb[0:1, s * 128 : (s + 1) * 128],
                                         ones[0:1, :], start=False, stop=s == 1)
                    dep(m, first)
                    lasts.append(m)
                dep(lasts[1], lasts[0])
                fo = interm.tile([P, 2, B], f32, tag="fo")
                nc.vector.tensor_copy(fo[:], pf[:])
                nc.sync.dma_start(out_d[0], fo[:, 0, :])
                nc.sync.dma_start(out_d[1], fo[:, 1, :])

            # prologue: load chunks 0,1; process L0 chunk 0 (no L1 yet)
            nc.sync.dma_start(xr[0][:], x_d[:, ds(0, CH * B)])
            nc.sync.dma_start(xr[1][:], x_d[:, ds(CH * B, CH * B)])
            emit_half(0, None, with_l1=False)

            # main loop: 7 iterations x 2 chunks; L0 chunks 1..14, L1 chunks 0..13
            if NCH_ == 16:
                with tc.For_i(0, 7) as i:
                    emit_half(1, i * (2 * CH * B) + 2 * CH * B, with_l1=True)
                    emit_half(0, i * (2 * CH * B) + 3 * CH * B, with_l1=True)
                # epilogue: L0 chunk 15 + L1 chunk 14; then L1 chunk 15; fc
                emit_half(1, None, with_l1=True)
                emit_half(0, None, with_l1=True, with_l0=False)
                fc_emit()
            else:
                # small-T debug variant: fully unrolled
                for c in range(1, NCH_):
                    emit_half(c % 2, (c + 1) * CH * B if c + 1 < NCH_ else None, with_l1=True)
                emit_half((NCH_) % 2, None, with_l1=True, with_l0=False)
                fc_emit()

    nc.compile()
    return nc


def _get_nc(T_=T):
    if T_ not in _NC_CACHE:
        _NC_CACHE[T_] = _build(T_)
    return _NC_CACHE[T_]


def _prep_inputs(x, W_ih0, W_hh0, b_ih0, b_hh0, W_ih1, W_hh1, b_ih1, b_hh1, fc_W, fc_b, T_=T):
    f16 = np.float16
    f32 = np.float32
    as32 = lambda a: np.asarray(a, dtype=f32)
    W_ih0, W_hh0, W_ih1, W_hh1, fc_W = map(as32, (W_ih0, W_hh0, W_ih1, W_hh1, fc_W))
    b_ih0, b_hh0, b_ih1, b_hh1, fc_b = map(as32, (b_ih0, b_hh0, b_ih1, b_hh1, fc_b))

    def dbl_T(Wt):  # -> lhsT [K, 768] with doubled n columns
        W = Wt.T.copy()
        W[:, 2 * H :] *= 2.0
        return W

    def fold2(Wl):  # [256, 768] -> [128, 1536]
        return np.concatenate([Wl[:128], Wl[128:]], axis=1)

    aug0 = np.concatenate(
        [b_ih0[: 2 * H] + b_hh0[: 2 * H] - W_hh0[: 2 * H].sum(1), 2.0 * b_ih0[2 * H :]]
    ).astype(f32)
    wih0_p = np.vstack([dbl_T(W_ih0), aug0[None]]).astype(f16)
    whh0_p = fold2(dbl_T(W_hh0)).astype(f16)
    whh1_p = fold2(dbl_T(W_hh1)).astype(f16)
    wih1_p = fold2(dbl_T(W_ih1)).astype(f16)

    def btile(vec):  # [256] gate-rows -> [128, 2*B] broadcast over batch
        return np.ascontiguousarray(
            np.repeat(vec.reshape(2, 128).T[:, :, None], B, axis=2).reshape(128, 2 * B)
        ).astype(f16)

    ident_p = np.eye(P, dtype=f16)
    bias0B_p = btile(2.0 * (b_hh0[2 * H :] - W_hh0[2 * H :].sum(1)))
    bias1B_p = btile(2.0 * (b_hh1[2 * H :] - W_hh1[2 * H :].sum(1)))
    b1A_vec = np.concatenate(
        [b_ih1[: 2 * H] + b_hh1[: 2 * H] - W_ih1[: 2 * H].sum(1) - W_hh1[: 2 * H].sum(1),
         2.0 * (b_ih1[2 * H :] - W_ih1[2 * H :].sum(1))]
    ).astype(f32)
    bias1A_p = np.ascontiguousarray(b1A_vec.reshape(6, 128).T)
    fcwT = fc_W.T.copy()
    fcw_p = np.concatenate([fcwT[:128], fcwT[128:]], axis=1).astype(f16)
    fcb_p = (fc_b - fc_W.sum(1)).astype(f16)[None]

    xf = np.asarray(x, dtype=f32).reshape(x.shape[0], T_, -1)
    in_maps = []
    for c in range(NCORES):
        xc = xf[c * B : (c + 1) * B]  # [32, T, 75]
        xp = np.empty((K0, T_ * B), f16)
        xp[:75] = xc.transpose(2, 1, 0).reshape(75, T_ * B).astype(f16)
        xp[75] = 1.0
        in_maps.append(dict(
            x=np.ascontiguousarray(xp), wih0=wih0_p, whh0=whh0_p, wih1=wih1_p,
            whh1=whh1_p, ident=ident_p, bias0B=bias0B_p, bias1A=bias1A_p,
            bias1B=bias1B_p, fcw=fcw_p, fcb=fcb_p))
    return in_maps


def kernel(x, W_ih0, W_hh0, b_ih0, b_hh0, W_ih1, W_hh1, b_ih1, b_hh1, fc_W, fc_b):
    from concourse import bass_utils

    in_maps = _prep_inputs(x, W_ih0, W_hh0, b_ih0, b_hh0, W_ih1, W_hh1,
                           b_ih1, b_hh1, fc_W, fc_b)
    nc = _get_nc()
    res = bass_utils.run_bass_kernel_spmd(nc, in_maps, core_ids=list(range(NCORES)))
    out = np.empty((x.shape[0], H), np.float32)
    for c in range(NCORES):
        o = res.results[c]["out"]  # [2, 128, 32]
        out[c * B : (c + 1) * B] = o.transpose(2, 0, 1).reshape(B, H)
    return out


# revision 30
# speedup vs baseline: 1.1828x; 1.1828x over previous
# nn_GRUEncoder: B=256, T=512, IN=75, H=256, 2-layer GRU + fc.
# Data-parallel over 8 NeuronCores (32 batch rows each). Full inputs in,
# full output out.
#
# Per-core Bass/Tile kernel design:
#  - All tensors live "transposed": hidden/gate dims on SBUF partitions,
#    batch (32) on the free dim. fp16 matmul operands, fp32 PSUM/DVE math.
#  - GRU state is stored offset: ht = h + 1 (so h0=0 -> ht=1). With
#    n = tanh(p) = 2*sigmoid(2p) - 1 and doubled n-gate weights, the whole
#    per-step elementwise chain is sigmoid-only:
#        d = ht - 2s        (= h - n)       [scalar_tensor_tensor]
#        e = z * d                          [tensor_tensor]
#        ht' = 2s + e       (= h' + 1)      [scalar_tensor_tensor]
#    The -rowsum(W) corrections from the +1 offset and all biases fold into
#    weight-augmentation rows / eviction biases at host-prep time.
#  - Layer-0 input projection xg0 runs per-step (aug row in x carries all
#    layer-0 psumA biases). Layer-1 input projection xg1 runs as a bulk
#    GEMM per 16-step sub-chunk into an SBUF ring (bf16/fp16, no DRAM).
#  - The two layers run as a staggered wavefront: while layer 0 processes
#    chunk c, layer 1 processes chunk c-1, so their serial chains hide
#    each other and every engine stays busy.

import sys

sys.path.insert(0, "/opt/trn_rl_repo")

import numpy as np

P, B, H, G, K0, CH, T = 128, 32, 256, 768, 76, 32, 512
NCH = T // CH
NCORES = 8

_NC_CACHE = {}


def _build(T_=T):
    import concourse.bass as bass
    import concourse.tile as tile
    from concourse import mybir
    from concourse.bass import ds, ts

    f16 = mybir.dt.float16
    f32 = mybir.dt.float32
    AF = mybir.ActivationFunctionType
    OP = mybir.AluOpType
    NCH_ = T_ // CH

    from concourse import bacc

    nc = bacc.Bacc(None, target_bir_lowering=False)
    x_d = nc.dram_tensor("x", [K0, T_ * B], f16, kind="ExternalInput")
    wih0_d = nc.dram_tensor("wih0", [K0, G], f16, kind="ExternalInput")
    whh0_d = nc.dram_tensor("whh0", [P, 2 * G], f16, kind="ExternalInput")
    wih1_d = nc.dram_tensor("wih1", [P, 2 * G], f16, kind="ExternalInput")
    whh1_d = nc.dram_tensor("whh1", [P, 2 * G], f16, kind="ExternalInput")
    ident_d = nc.dram_tensor("ident", [P, P], f16, kind="ExternalInput")
    bias0B_d = nc.dram_tensor("bias0B", [P, 2 * B], f16, kind="ExternalInput")
    bias1A_d = nc.dram_tensor("bias1A", [P, 6], f32, kind="ExternalInput")
    bias1B_d = nc.dram_tensor("bias1B", [P, 2 * B], f16, kind="ExternalInput")
    fcw_d = nc.dram_tensor("fcw", [P, 2 * H], f16, kind="ExternalInput")
    fcb_d = nc.dram_tensor("fcb", [1, H], f16, kind="ExternalInput")
    out_d = nc.dram_tensor("out", [2, P, B], f32, kind="ExternalOutput")

    with tile.TileContext(nc) as tc:
        from contextlib import ExitStack

        with ExitStack() as ctx:
            consts = ctx.enter_context(tc.tile_pool(name="consts", bufs=1))
            interm = ctx.enter_context(tc.tile_pool(name="interm", bufs=6))
            # PSUM: 8 banks total. Layer0: R/ZN/B banks (bufs=1 each);
            # Layer1: R/Z/B banks (bufs=1); bulk GEMM: 2 banks.
            psL0 = ctx.enter_context(tc.tile_pool(name="psL0", bufs=1, space="PSUM"))
            psL1 = ctx.enter_context(tc.tile_pool(name="psL1", bufs=1, space="PSUM"))
            psK = ctx.enter_context(tc.tile_pool(name="psK", bufs=2, space="PSUM"))

            def dep(a, b):
                # order-only edge: a must execute after b (same engine)
                tile.add_dep_helper(a.ins, b.ins, sync=False, reason="psum-group-order")

            wih0 = consts.tile([K0, G], f16)
            whh0 = consts.tile([P, 2 * G], f16)
            wih1 = consts.tile([P, 2 * G], f16)
            whh1 = consts.tile([P, 2 * G], f16)
            ident = consts.tile([P, P], f16)
            bias0B = consts.tile([P, 2, B], f16)
            bias1A = consts.tile([P, 6], f32)
            bias1B = consts.tile([P, 2, B], f16)
            fcw = consts.tile([P, 2 * H], f16)
            fcb = consts.tile([1, H], f16)
            nc.sync.dma_start(wih0[:], wih0_d[:])
            nc.sync.dma_start(whh0[:], whh0_d[:])
            nc.sync.dma_start(wih1[:], wih1_d[:])
            nc.sync.dma_start(whh1[:], whh1_d[:])
            nc.sync.dma_start(ident[:], ident_d[:])
            nc.sync.dma_start(bias0B[:], bias0B_d[:].rearrange("p (s b) -> p s b", b=B))
            nc.sync.dma_start(bias1A[:], bias1A_d[:])
            nc.sync.dma_start(bias1B[:], bias1B_d[:].rearrange("p (s b) -> p s b", b=B))
            nc.sync.dma_start(fcw[:], fcw_d[:])
            nc.sync.dma_start(fcb[:], fcb_d[:])

            ones = consts.tile([1, B], f16)
            nc.vector.memset(ones[:], 1.0)

            # x chunk staging ring (2 slots), per-chunk DMA from DRAM
            xr = [consts.tile([K0, CH * B], f16, tag=f"xr{i}", name=f"xr{i}") for i in range(2)]
            # layer-0 state ring: ht0 per step, [k-chunk, t, b]; doubles as
            # the moving operand for the bulk xg1 GEMM
            r0 = [consts.tile([P, 2, CH, B], f16, tag=f"r0{i}", name=f"r0{i}") for i in range(2)]
            # xg1 ring: [t, strip, b]
            r1 = [consts.tile([P, CH, 6, B], f16, tag=f"r1{i}", name=f"r1{i}") for i in range(2)]
            h1 = consts.tile([P, 2, B], f16)

            # ht init = 1 (h=0). Chunk 0 step 0 reads r0[1][:, :, CH-1, :].
            nc.vector.memset(r0[1][:, :, CH - 1, :], 1.0)
            nc.vector.memset(h1[:], 1.0)

            def l0_step(t, par, hp, hout):
                """One layer-0 GRU step. hp: [P,2,B] ht_{t-1}; hout: ring slot."""
                bR = psL0.tile([P, 2, B], f32, tag="bR0")   # pre-r (biased)
                bZN = psL0.tile([P, 4, B], f32, tag="bZN0")  # pre-z, 2*xn -> npre2
                bB = psL0.tile([P, 2, B], f32, tag="bB0")   # 2*(hn + b_hn)
                xs = xr[par][:, ts(t, B)]
                # --- bank R: 6 MMs, one group ---
                m0 = nc.tensor.matmul(bR[:, 0, :], wih0[:, 0:128], xs, start=True, stop=False)
                m1 = nc.tensor.matmul(bR[:, 1, :], wih0[:, 128:256], xs, start=False, stop=False)
                dep(m1, m0)
                last0 = last1 = None
                for k in (0, 1):
                    hk = hp[:, k, :]
                    last0 = nc.tensor.matmul(bR[:, 0, :], whh0[:, k * G + 0 : k * G + 128], hk, start=False, stop=False)
                    last1 = nc.tensor.matmul(bR[:, 1, :], whh0[:, k * G + 128 : k * G + 256], hk, start=False, stop=k == 1)
                dep(last1, last0)
                r = interm.tile([P, 2, B], f16, tag="r")
                nc.scalar.activation(r[:], bR[:], AF.Sigmoid)
                # --- bank ZN: z strips 0-1, n strips 2-3; one group ---
                z0 = nc.tensor.matmul(bZN[:, 0, :], wih0[:, 256:384], xs, start=True, stop=False)
                z1 = nc.tensor.matmul(bZN[:, 1, :], wih0[:, 384:512], xs, start=False, stop=False)
                n0 = nc.tensor.matmul(bZN[:, 2, :], wih0[:, 512:640], xs, start=False, stop=False)
                n1 = nc.tensor.matmul(bZN[:, 3, :], wih0[:, 640:768], xs, start=False, stop=False)
                for m in (z1, n0, n1):
                    dep(m, z0)
                lz0 = lz1 = None
                for k in (0, 1):
                    hk = hp[:, k, :]
                    lz0 = nc.tensor.matmul(bZN[:, 0, :], whh0[:, k * G + 256 : k * G + 384], hk, start=False, stop=False)
                    lz1 = nc.tensor.matmul(bZN[:, 1, :], whh0[:, k * G + 384 : k * G + 512], hk, start=False, stop=k == 1)
                for m in (lz0, n0, n1):
                    dep(lz1, m)
                # --- bank B: bias seed (identity matmul) first, then hg-n ---
                nc.tensor.matmul(bB[:], ident[:], bias0B[:], start=True, stop=False)
                lb0 = lb1 = None
                for k in (0, 1):
                    hk = hp[:, k, :]
                    lb0 = nc.tensor.matmul(bB[:, 0, :], whh0[:, k * G + 512 : k * G + 640], hk, start=False, stop=False)
                    lb1 = nc.tensor.matmul(bB[:, 1, :], whh0[:, k * G + 640 : k * G + 768], hk, start=False, stop=k == 1)
                dep(lb1, lb0)
                t1 = interm.tile([P, 2, B], f16, tag="t1")
                nc.vector.tensor_tensor(t1[:], r[:], bB[:], OP.mult)
                nc.vector.tensor_tensor(bZN[:, 2:4, :], bZN[:, 2:4, :], t1[:], OP.add)
                szn = interm.tile([P, 4, B], f16, tag="szn")
                nc.scalar.activation(szn[:], bZN[:], AF.Sigmoid)
                d = interm.tile([P, 2, B], f16, tag="d")
                nc.vector.scalar_tensor_tensor(d[:], szn[:, 2:4, :], -2.0, hp, OP.mult, OP.add)
                e = interm.tile([P, 2, B], f16, tag="e")
                nc.vector.tensor_tensor(e[:], szn[:, 0:2, :], d[:], OP.mult)
                nc.vector.scalar_tensor_tensor(hout, szn[:, 2:4, :], 2.0, e[:], OP.mult, OP.add)

            def l1_step(t, par):
                """One layer-1 GRU step. xg1 + biases enter the step PSUM banks
                via identity-matmul seeds (cheap, dependency-free PE work);
                hg accumulates on top. State h1 updated in place."""
                bR = psL1.tile([P, 2, B], f32, tag="bR1")
                bZN = psL1.tile([P, 4, B], f32, tag="bZN1")
                bB = psL1.tile([P, 2, B], f32, tag="bB1")
                rg = r1[par]
                # seeds (full-bank writes; later strip MMs auto-order after them)
                nc.tensor.matmul(bR[:], ident[:], rg[:, t, 0:2, :], start=True, stop=False)
                nc.tensor.matmul(bZN[:], ident[:], rg[:, t, 2:6, :], start=True, stop=False)
                nc.tensor.matmul(bB[:], ident[:], bias1B[:], start=True, stop=False)
                # bank R: + hg-r
                l0_ = l1_ = None
                for k in (0, 1):
                    hk = h1[:, k, :]
                    l0_ = nc.tensor.matmul(bR[:, 0, :], whh1[:, k * G + 0 : k * G + 128], hk, start=False, stop=False)
                    l1_ = nc.tensor.matmul(bR[:, 1, :], whh1[:, k * G + 128 : k * G + 256], hk, start=False, stop=k == 1)
                dep(l1_, l0_)
                r = interm.tile([P, 2, B], f16, tag="r1t")
                nc.scalar.activation(r[:], bR[:], AF.Sigmoid)
                # bank B: + hg-n
                lb0 = lb1 = None
                for k in (0, 1):
                    hk = h1[:, k, :]
                    lb0 = nc.tensor.matmul(bB[:, 0, :], whh1[:, k * G + 512 : k * G + 640], hk, start=False, stop=False)
                    lb1 = nc.tensor.matmul(bB[:, 1, :], whh1[:, k * G + 640 : k * G + 768], hk, start=False, stop=k == 1)
                dep(lb1, lb0)
                # bank ZN: + hg-z into strips 0-1
                lz0 = lz1 = None
                for k in (0, 1):
                    hk = h1[:, k, :]
                    lz0 = nc.tensor.matmul(bZN[:, 0, :], whh1[:, k * G + 256 : k * G + 384], hk, start=False, stop=False)
                    lz1 = nc.tensor.matmul(bZN[:, 1, :], whh1[:, k * G + 384 : k * G + 512], hk, start=False, stop=k == 1)
                dep(lz1, lz0)
                t1 = interm.tile([P, 2, B], f16, tag="t1b")
                nc.vector.tensor_tensor(t1[:], r[:], bB[:], OP.mult)
                nc.vector.tensor_tensor(bZN[:, 2:4, :], bZN[:, 2:4, :], t1[:], OP.add)
                szn = interm.tile([P, 4, B], f16, tag="szn1")
                nc.scalar.activation(szn[:], bZN[:], AF.Sigmoid)
                d = interm.tile([P, 2, B], f16, tag="d1")
                nc.vector.scalar_tensor_tensor(d[:], szn[:, 2:4, :], -2.0, h1[:, :, :], OP.mult, OP.add)
                e = interm.tile([P, 2, B], f16, tag="e1")
                nc.vector.tensor_tensor(e[:], szn[:, 0:2, :], d[:], OP.mult)
                nc.vector.scalar_tensor_tensor(h1[:, :, :], szn[:, 2:4, :], 2.0, e[:], OP.mult, OP.add)

            def bulk_pieces(par, u):
                """xg1 bulk GEMM for 16 steps (sub-chunk u of the chunk in
                ring slot par): 6 strips x (2 matmuls + biased evict).
                Returns a list of closures to spread across step emission."""
                ops = []
                for s in range(6):
                    hold = {}

                    def mm0(s=s, hold=hold):
                        bp = psK.tile([P, 512], f32, tag="bp")
                        hold["bp"] = bp
                        nc.tensor.matmul(
                            bp[:], wih1[:, s * 128 : (s + 1) * 128],
                            r0[par][:, 0, u * 16 : (u + 1) * 16, :], start=True, stop=False)

                    def mm1(s=s, hold=hold):
                        bp = hold["bp"]
                        nc.tensor.matmul(
                            bp[:], wih1[:, G + s * 128 : G + (s + 1) * 128],
                            r0[par][:, 1, u * 16 : (u + 1) * 16, :], start=False, stop=True)

                    def ev(s=s, hold=hold):
                        bp = hold["bp"]
                        nc.vector.tensor_scalar(
                            r1[par][:, u * 16 : (u + 1) * 16, s, :],
                            bp[:].rearrange("p (t b) -> p t b", b=B),
                            bias1A[:, s : s + 1], None, OP.add)

                    ops += [mm0, mm1, ev]
                return ops

            def emit_half(par, xoff_next, with_l1, with_l0=True, last_l1_par=None):
                """Process L0 chunk (parity par) + L1 chunk (parity 1-par)."""
                if xoff_next is not None:
                    nc.sync.dma_start(xr[1 - par][:], x_d[:, ds(xoff_next, CH * B)])
                u1p = bulk_pieces(1 - par, 1) if with_l1 else []
                u0p = bulk_pieces(par, 0) if with_l0 else []
                n1, n0 = len(u1p), len(u0p)
                for t in range(CH):
                    if with_l0:
                        hp = r0[par][:, :, t - 1, :] if t > 0 else r0[1 - par][:, :, CH - 1, :]
                        l0_step(t, par, hp, r0[par][:, :, t, :])
                    if with_l1:
                        l1_step(t, 1 - par)
                    # spread prev-chunk sub-1 bulk over t=0..15
                    if t < 16:
                        for i in range(t * n1 // 16, (t + 1) * n1 // 16):
                            u1p[i]()
                    else:
                        for i in range((t - 16) * n0 // 16, (t - 15) * n0 // 16):
                            u0p[i]()

            def fc_emit():
                pf = psK.tile([P, 2, B], f32, tag="bp")
                first = None
                lasts = []
                for s in (0, 1):
                    for k in (0, 1):
                        m = nc.tensor.matmul(
                            pf[:, s, :], fcw[:, k * H + s * 128 : k * H + (s + 1) * 128],
                            h1[:, k, :], start=first is None, stop=False)
                        if first is None:
                            first = m
                        else:
                            dep(m, first)
                    m = nc.tensor.matmul(pf[:, s, :], fcb[0:1, s * 128 : (s + 1) * 128],
                                         ones[0:1, :], start=False, stop=s == 1)
                    dep(m, first)
                    lasts.append(m)
                dep(lasts[1], lasts[0])
                fo = interm.tile([P, 2, B], f32, tag="fo")
                nc.vector.tensor_copy(fo[:], pf[:])
                nc.sync.dma_start(out_d[0], fo[:, 0, :])
                nc.sync.dma_start(out_d[1], fo[:, 1, :])

            # prologue: load chunks 0,1; process L0 chunk 0 (no L1 yet)
            nc.sync.dma_start(xr[0][:], x_d[:, ds(0, CH * B)])
            nc.sync.dma_start(xr[1][:], x_d[:, ds(CH * B, CH * B)])
            emit_half(0, None, with_l1=False)

            # main loop: 7 iterations x 2 chunks; L0 chunks 1..14, L1 chunks 0..13
            if NCH_ == 16:
                with tc.For_i(0, 7) as i:
                    emit_half(1, i * (2 * CH * B) + 2 * CH * B, with_l1=True)
                    emit_half(0, i * (2 * CH * B) + 3 * CH * B, with_l1=True)
                # epilogue: L0 chunk 15 + L1 chunk 14; then L1 chunk 15; fc
                emit_half(1, None, with_l1=True)
                emit_half(0, None, with_l1=True, with_l0=False)
                fc_emit()
            else:
                # small-T debug variant: fully unrolled
                for c in range(1, NCH_):
                    emit_half(c % 2, (c + 1) * CH * B if c + 1 < NCH_ else None, with_l1=True)
                emit_half((NCH_) % 2, None, with_l1=True, with_l0=False)
                fc_emit()

    nc.compile()
    return nc


def _get_nc(T_=T):
    if T_ not in _NC_CACHE:
        _NC_CACHE[T_] = _build(T_)
    return _NC_CACHE[T_]


def _prep_inputs(x, W_ih0, W_hh0, b_ih0, b_hh0, W_ih1, W_hh1, b_ih1, b_hh1, fc_W, fc_b, T_=T):
    f16 = np.float16
    f32 = np.float32
    as32 = lambda a: np.asarray(a, dtype=f32)
    W_ih0, W_hh0, W_ih1, W_hh1, fc_W = map(as32, (W_ih0, W_hh0, W_ih1, W_hh1, fc_W))
    b_ih0, b_hh0, b_ih1, b_hh1, fc_b = map(as32, (b_ih0, b_hh0, b_ih1, b_hh1, fc_b))

    def dbl_T(Wt):  # -> lhsT [K, 768] with doubled n columns
        W = Wt.T.copy()
        W[:, 2 * H :] *= 2.0
        return W

    def fold2(Wl):  # [256, 768] -> [128, 1536]
        return np.concatenate([Wl[:128], Wl[128:]], axis=1)

    aug0 = np.concatenate(
        [b_ih0[: 2 * H] + b_hh0[: 2 * H] - W_hh0[: 2 * H].sum(1), 2.0 * b_ih0[2 * H :]]
    ).astype(f32)
    wih0_p = np.vstack([dbl_T(W_ih0), aug0[None]]).astype(f16)
    whh0_p = fold2(dbl_T(W_hh0)).astype(f16)
    whh1_p = fold2(dbl_T(W_hh1)).astype(f16)
    wih1_p = fold2(dbl_T(W_ih1)).astype(f16)

    def btile(vec):  # [256] gate-rows -> [128, 2*B] broadcast over batch
        return np.ascontiguousarray(
            np.repeat(vec.reshape(2, 128).T[:, :, None], B, axis=2).reshape(128, 2 * B)
        ).astype(f16)

    ident_p = np.eye(P, dtype=f16)
    bias0B_p = btile(2.0 * (b_hh0[2 * H :] - W_hh0[2 * H :].sum(1)))
    bias1B_p = btile(2.0 * (b_hh1[2 * H :] - W_hh1[2 * H :].sum(1)))
    b1A_vec = np.concatenate(
        [b_ih1[: 2 * H] + b_hh1[: 2 * H] - W_ih1[: 2 * H].sum(1) - W_hh1[: 2 * H].sum(1),
         2.0 * (b_ih1[2 * H :] - W_ih1[2 * H :].sum(1))]
    ).astype(f32)
    bias1A_p = np.ascontiguousarray(b1A_vec.reshape(6, 128).T)
    fcwT = fc_W.T.copy()
    fcw_p = np.concatenate([fcwT[:128], fcwT[128:]], axis=1).astype(f16)
    fcb_p = (fc_b - fc_W.sum(1)).astype(f16)[None]

    xf = np.asarray(x, dtype=f32).reshape(x.shape[0], T_, -1)
    in_maps = []
    for c in range(NCORES):
        xc = xf[c * B : (c + 1) * B]  # [32, T, 75]
        xp = np.empty((K0, T_ * B), f16)
        xp[:75] = xc.transpose(2, 1, 0).reshape(75, T_ * B).astype(f16)
        xp[75] = 1.0
        in_maps.append(dict(
            x=np.ascontiguousarray(xp), wih0=wih0_p, whh0=whh0_p, wih1=wih1_p,
            whh1=whh1_p, ident=ident_p, bias0B=bias0B_p, bias1A=bias1A_p,
            bias1B=bias1B_p, fcw=fcw_p, fcb=fcb_p))
    return in_maps


def kernel(x, W_ih0, W_hh0, b_ih0, b_hh0, W_ih1, W_hh1, b_ih1, b_hh1, fc_W, fc_b):
    from concourse import bass_utils

    in_maps = _prep_inputs(x, W_ih0, W_hh0, b_ih0, b_hh0, W_ih1, W_hh1,
                           b_ih1, b_hh1, fc_W, fc_b)
    nc = _get_nc()
    res = bass_utils.run_bass_kernel_spmd(nc, in_maps, core_ids=list(range(NCORES)))
    out = np.empty((x.shape[0], H), np.float32)
    for c in range(NCORES):
        o = res.results[c]["out"]  # [2, 128, 32]
        out[c * B : (c + 1) * B] = o.transpose(2, 0, 1).reshape(B, H)
    return out
